# revision 1
# baseline (speedup 1.0000x reference)
"""Trainium2 Bass kernel for nn_CTRule (temporal KG scoring model).

Computes, for each of B=1024 queries (h, r, t):
  v = f(E0[h], E1[r], time tables, rule tables)   # [B, 128] elementwise algebra
  scores = v @ E0.T                               # [B, 40000]

Distribution over the 8 NeuronCores (pair-hybrid): the two cores sharing each
HBM stack process the same PAIR of 128-example batch tiles but disjoint halves
of the 40000-entity axis, so each stack streams the bf16 entity table E0T only
half as much as a fully data-parallel split would:
  * each core gathers (indirect DMA) both batch tiles' embedding rows and runs
    one batched elementwise head over [128, 2, 128] on VectorE, transposing
    each v on TensorE,
  * streams its half of E0T (20000 cols, 5.12MB bf16) and matmuls both vT
    blocks against it (paired matmuls share one [128,1024] PSUM tile so each
    PSUM->SBUF copy moves 1024 columns),
  * writes a [256, 20000] bf16 block of the scores.
No cross-core communication; the host reassembles the 8 blocks and casts f32.
"""

import numpy as np
import ml_dtypes

P = 128
B = 1024
RANK = 128
NENT = 40000
NREL = 230
NTIME = 365
NBASE = 4
CYCLE = 120
NCORES = 8
NHALF = NENT // 2        # entity columns per core = 20000
CHUNK = 512              # PSUM bank width in f32
PAIRW = 2 * CHUNK        # paired-matmul copy width
LOADCH = 5000            # E0T load-chunk columns (4 loads of 1.28MB)
OUTCH = 2500             # output DMA chunk columns (8 DMAs of 640KB per tile)

# concatenated-table row widths
RC_W = 2 * RANK + 2      # [E1 | rule_C | rule_S | has_rules] = 258
TC_W = 3 * RANK          # [E4 | E2 | E3] = 384
BC_W = 2 * RANK          # [E5 | E6] = 256

TRACE = False            # set by test harness for profiling runs
_CACHE = {}


def _build():
    import concourse.bass as bass
    import concourse.mybir as mybir
    import concourse.tile as tile
    from concourse import bacc
    from concourse.masks import make_identity

    dt = mybir.dt
    mult = mybir.AluOpType.mult
    add = mybir.AluOpType.add
    sub = mybir.AluOpType.subtract

    nc = bacc.Bacc("TRN2", target_bir_lowering=False, debug=False,
                   num_devices=NCORES)

    IDX = nc.dram_tensor("IDX", [P, 8], dt.int32, kind="ExternalInput").ap()
    E0 = nc.dram_tensor("E0", [NENT, RANK], dt.float32, kind="ExternalInput").ap()
    RCAT = nc.dram_tensor("RCAT", [NREL, RC_W], dt.float32, kind="ExternalInput").ap()
    TCAT = nc.dram_tensor("TCAT", [NTIME, TC_W], dt.float32, kind="ExternalInput").ap()
    BCAT = nc.dram_tensor("BCAT", [NBASE, BC_W], dt.float32, kind="ExternalInput").ap()
    E0T = nc.dram_tensor("E0T", [RANK, NHALF], dt.bfloat16, kind="ExternalInput").ap()
    OUT = nc.dram_tensor("OUT", [2 * P, NHALF], dt.bfloat16, kind="ExternalOutput").ap()

    with tile.TileContext(nc) as tc:
        with (
            tc.tile_pool(name="const", bufs=1) as constp,
            tc.tile_pool(name="gath", bufs=1) as gp,
            tc.tile_pool(name="ew", bufs=1) as ew,
            tc.tile_pool(name="pst", bufs=1, space="PSUM") as pst,
            tc.tile_pool(name="psm", bufs=6, space="PSUM") as psm,
        ):
            # ---- index load on gpsimd's own SWDGE path so the gather chain
            # never waits on another engine's DMA ring
            idxt = gp.tile([P, 8], dt.int32)
            nc.gpsimd.dma_start(idxt[:], IDX[:])

            # ---- 4 indirect gathers per batch tile into [P, 2, W] tiles,
            # interleaved with the E0T half-table stream on the same SWDGE
            # ring: tile 0's gather packets drain first, then the first E0T
            # chunks, then tile 1's gathers, then the rest of E0T.
            lhs = gp.tile([P, 2, RANK], dt.float32)
            r8 = gp.tile([P, 2, RC_W], dt.float32)
            t8 = gp.tile([P, 2, TC_W], dt.float32)
            b8 = gp.tile([P, 2, BC_W], dt.float32)
            e0t = constp.tile([RANK, NHALF], dt.bfloat16)
            for j in range(2):
                for dst, src, col in (
                    (t8, TCAT, 2), (b8, BCAT, 3), (r8, RCAT, 1), (lhs, E0, 0),
                ):
                    nc.gpsimd.indirect_dma_start(
                        out=dst[:, j, :], out_offset=None, in_=src[:],
                        in_offset=bass.IndirectOffsetOnAxis(
                            ap=idxt[:, 4 * j + col:4 * j + col + 1],
                            axis=0))
            for c0 in range(0, NHALF, LOADCH):
                nc.gpsimd.dma_start(e0t[:, c0:c0 + LOADCH],
                                    E0T[:, c0:c0 + LOADCH])

            ident = constp.tile([P, P], dt.float32)
            make_identity(nc, ident[:])

            def TT(out, a, b_, op):
                nc.vector.tensor_tensor(out=out, in0=a, in1=b_, op=op)

            def h0(x):
                return x[:, :, 0:64]

            def h1(x):
                return x[:, :, 64:128]

            # ---- per-tile elementwise head (tile 0 first, so its matmuls
            # and output stream start as early as possible)
            TM = ew.tile([P, RANK], dt.float32)
            TE = ew.tile([P, RANK], dt.float32)
            A = ew.tile([P, RANK], dt.float32)
            Bt = ew.tile([P, RANK], dt.float32)
            Sg = ew.tile([P, RANK], dt.float32)
            Dg = ew.tile([P, RANK], dt.float32)
            t0 = ew.tile([P, 64], dt.float32)
            t1 = ew.tile([P, 64], dt.float32)
            nrs = ew.tile([P, 1], dt.float32)

            vts = []
            for j in range(2):
                REL = r8[:, j, 0:128]
                RL0, RL1 = r8[:, j, 0:64], r8[:, j, 64:128]
                RC0, RC1 = r8[:, j, 128:192], r8[:, j, 192:256]
                RS = r8[:, j, 256:257]
                HR = r8[:, j, 257:258]
                CT = t8[:, j, 0:128]
                CT0, CT1 = t8[:, j, 0:64], t8[:, j, 64:128]
                E2g, B5 = t8[:, j, 128:256], b8[:, j, 0:128]
                E3g, B6 = t8[:, j, 256:384], b8[:, j, 128:256]
                LHS = lhs[:, j, 0:128]
                L0, L1 = lhs[:, j, 0:64], lhs[:, j, 64:128]
                V = ew.tile([P, RANK], dt.float32, name=f"V{j}")

                def g0(x):
                    return x[:, 0:64]

                def g1(x):
                    return x[:, 64:128]

                # time = E2[t] + E5[tb];  time_ent = E3[t] + E6[tb]
                TT(TM[:], E2g, B5, add)
                TT(TE[:], E3g, B6, add)
                nc.vector.tensor_scalar_mul(nrs[:], RS, -1.0)
                # A = cmul(comp_time, rule_C[r])
                TT(t0[:], CT0, RC0, mult)
                TT(t1[:], CT1, RC1, mult)
                TT(g0(A), t0[:], t1[:], sub)
                TT(t0[:], CT0, RC1, mult)
                TT(t1[:], CT1, RC0, mult)
                TT(g1(A), t0[:], t1[:], add)
                # A = rule_branch = A - rule_S*rel (fused: A = rel*(-RS) + A)
                nc.vector.scalar_tensor_tensor(
                    out=A[:], in0=REL, scalar=nrs[:], in1=A[:],
                    op0=mult, op1=add)
                # Bt = cmul(rel, lhs); then Bt = norule_branch = lhs + Bt
                TT(t0[:], RL0, L0, mult)
                TT(t1[:], RL1, L1, mult)
                TT(g0(Bt), t0[:], t1[:], sub)
                TT(t0[:], RL0, L1, mult)
                TT(t1[:], RL1, L0, mult)
                TT(g1(Bt), t0[:], t1[:], add)
                TT(Bt[:], LHS, Bt[:], add)
                # A = rule_score = Bt + HR*(A - Bt); then A = q = CT + A
                TT(A[:], A[:], Bt[:], sub)
                nc.vector.scalar_tensor_tensor(
                    out=A[:], in0=A[:], scalar=HR, in1=Bt[:],
                    op0=mult, op1=add)
                TT(A[:], A[:], CT, add)
                # Bt = complex_mul(rel, q) = [R0*q0 + R1*q1, R0*q1 - R1*q0]
                TT(t0[:], RL0, g0(A), mult)
                TT(t1[:], RL1, g1(A), mult)
                TT(g0(Bt), t0[:], t1[:], add)
                TT(t0[:], RL0, g1(A), mult)
                TT(t1[:], RL1, g0(A), mult)
                TT(g1(Bt), t0[:], t1[:], sub)
                # Bt = rel_ = rel + Bt ;  S = rel_ + time ; D = rel_ - time
                TT(Bt[:], Bt[:], REL, add)
                TT(Sg[:], Bt[:], TM[:], add)
                TT(Dg[:], Bt[:], TM[:], sub)
                # V0 = L0*S0 + TE0*D0 - L1*S1 + TE1*D1
                TT(t0[:], L0, g0(Sg), mult)
                TT(t1[:], g0(TE), g0(Dg), mult)
                TT(g0(V), t0[:], t1[:], add)
                TT(t0[:], L1, g1(Sg), mult)
                TT(g0(V), g0(V), t0[:], sub)
                TT(t1[:], g1(TE), g1(Dg), mult)
                TT(g0(V), g0(V), t1[:], add)
                # V1 = L1*S0 + L0*S1 + TE1*D0 - TE0*D1
                TT(t0[:], L1, g0(Sg), mult)
                TT(t1[:], L0, g1(Sg), mult)
                TT(g1(V), t0[:], t1[:], add)
                TT(t0[:], g1(TE), g0(Dg), mult)
                TT(g1(V), g1(V), t0[:], add)
                TT(t1[:], g0(TE), g1(Dg), mult)
                TT(g1(V), g1(V), t1[:], sub)

                # transpose + bf16 cast of vT (stationary matmul operand)
                vt_ps = pst.tile([P, P], dt.float32, space="PSUM", tag="vtps")
                nc.tensor.transpose(out=vt_ps[:], in_=V[:], identity=ident[:])
                vt = constp.tile([P, P], dt.bfloat16, name=f"vt{j}")
                nc.scalar.copy(out=vt[:], in_=vt_ps[:])
                vts.append(vt)

            # ---- stream matmuls: both batch tiles over this core's half.
            # Pairs of 512-wide matmuls share one [128,1024] PSUM tile so each
            # PSUM->SBUF copy moves 1024 columns.
            for j in range(2):
                vt = vts[j]
                osb = constp.tile([P, NHALF], dt.bfloat16, name=f"osb{j}")
                next_out = OUTCH
                for c0 in range(0, NHALF, CHUNK):
                    cw = min(CHUNK, NHALF - c0)
                    mm = psm.tile([P, CHUNK], dt.float32, space="PSUM", tag="mm")
                    nc.tensor.matmul(out=mm[:, :cw], lhsT=vt[:],
                                     rhs=e0t[:, c0:c0 + cw],
                                     start=True, stop=True)
                    if (c0 // CHUNK) % 2 == 0:
                        nc.scalar.copy(out=osb[:, c0:c0 + cw], in_=mm[:, :cw])
                    else:
                        nc.vector.tensor_copy(out=osb[:, c0:c0 + cw],
                                              in_=mm[:, :cw])
                    if c0 + cw >= next_out:
                        o0 = next_out - OUTCH
                        nc.sync.dma_start(OUT[j * P:(j + 1) * P, o0:next_out],
                                          osb[:, o0:next_out])
                        next_out += OUTCH

    nc.compile()
    return nc


def _prep_inputs(inputs):
    x = np.asarray(inputs["x"])
    E0 = np.ascontiguousarray(np.asarray(inputs["E0"], dtype=np.float32))
    E1 = np.asarray(inputs["E1"], dtype=np.float32)
    E2 = np.asarray(inputs["E2"], dtype=np.float32)
    E3 = np.asarray(inputs["E3"], dtype=np.float32)
    E4 = np.asarray(inputs["E4"], dtype=np.float32)
    E5 = np.asarray(inputs["E5"], dtype=np.float32)
    E6 = np.asarray(inputs["E6"], dtype=np.float32)
    rule_C = np.asarray(inputs["rule_C"], dtype=np.float32)
    rule_S = np.asarray(inputs["rule_S"], dtype=np.float32)
    has_rules = np.asarray(inputs["has_rules"])

    idx = np.empty((B, 4), np.int32)
    idx[:, 0] = x[:, 0]
    idx[:, 1] = x[:, 1]
    idx[:, 2] = x[:, 3]
    idx[:, 3] = x[:, 3] // CYCLE

    rcat = np.concatenate(
        [E1, rule_C, rule_S[:, None],
         has_rules.astype(np.float32)[:, None]], axis=1).astype(np.float32)
    tcat = np.ascontiguousarray(np.concatenate([E4, E2, E3], axis=1))
    bcat = np.ascontiguousarray(np.concatenate([E5, E6], axis=1))
    e0t = np.ascontiguousarray(E0.T).astype(ml_dtypes.bfloat16)
    e0t_halves = [np.ascontiguousarray(e0t[:, :NHALF]),
                  np.ascontiguousarray(e0t[:, NHALF:])]

    in_maps = []
    for c in range(NCORES):
        p = c // 2
        idx2 = np.hstack([idx[2 * p * P:(2 * p + 1) * P],
                          idx[(2 * p + 1) * P:(2 * p + 2) * P]])
        in_maps.append({
            "IDX": np.ascontiguousarray(idx2),
            "E0": E0, "RCAT": rcat, "TCAT": tcat, "BCAT": bcat,
            "E0T": e0t_halves[c % 2],
        })
    return in_maps


def kernel(**inputs):
    from concourse.bass_utils import run_bass_kernel_spmd

    if "nc" not in _CACHE:
        _CACHE["nc"] = _build()
    nc = _CACHE["nc"]

    in_maps = _prep_inputs(inputs)
    res = run_bass_kernel_spmd(nc, in_maps, core_ids=list(range(NCORES)),
                               trace=TRACE)
    _CACHE["last_result"] = res
    out = np.empty((B, NENT), np.float32)
    for p in range(NCORES // 2):
        lo = res.results[2 * p]["OUT"]        # [256, 0:20000]
        hi = res.results[2 * p + 1]["OUT"]    # [256, 20000:40000]
        rows = slice(2 * p * P, (2 * p + 2) * P)
        out[rows, :NHALF] = lo
        out[rows, NHALF:] = hi
    return out



# revision 9
# speedup vs baseline: 1.0250x; 1.0250x over previous
"""Trainium2 Bass kernel for nn_CTRule (temporal KG scoring model).

Computes, for each of B=1024 queries (h, r, t):
  v = f(E0[h], E1[r], time tables, rule tables)   # [B, 128] elementwise algebra
  scores = v @ E0.T                               # [B, 40000]

Distribution over the 8 NeuronCores (pair-hybrid): the two cores of pair p
process batch tiles 2p, 2p+1 against disjoint halves of the 40000-entity
axis.  Per core: gather (indirect DMA) the three per-example table rows,
run the elementwise head in fp16 on VectorE+GpSimd, transpose v on TensorE,
stream this core's E0T half (fp16, 5.12MB) through 40-chunk matmuls per
tile, and write the fp16 [256, 20000] score block back to HBM.

Overlap structure (the point of this version vs the serialized baseline):
  * E0T chunk loads go on the Activation-engine HWDGE ring starting at t=0
    (no dependency on the index load).
  * idx goes on the Sync HWDGE ring first; gathers follow on gpsimd SWDGE.
  * OUT write groups go on the Sync ring as soon as each 2000-col group of
    PSUM->SBUF copies lands, so the 30us write stream overlaps the reads.
  * PSUM->SBUF copies are spread across Scalar/GpSimd/Vector.
Everything in the head is fp16 (2x DVE throughput; rel err ~1e-3 vs the
2e-2 gate).  No cross-core communication; the host reassembles 8 blocks.
"""

import numpy as np

P = 128
B = 1024
RANK = 128
NENT = 40000
NREL = 230
NTIME = 365
CYCLE = 120
NCORES = 8
NHALF = NENT // 2        # entity columns per core = 20000
CHUNK = 512              # matmul chunk columns (= one PSUM bank of f32)
LOADCH = 2500            # E0T load-chunk columns (8 loads of 0.64MB)
OUTCH = 2048             # output DMA group columns (2 copy groups)

RC_W = 2 * RANK + 2      # [E1 | rule_C | -rule_S | has_rules] = 258
TC_W = 3 * RANK          # [E4 | E2+E5b | E3+E6b] = 384

TRACE = False            # set by test harness for profiling runs
_CACHE = {}


def _build():
    import concourse.bass as bass
    import concourse.mybir as mybir
    import concourse.tile as tile
    from concourse import bacc
    from concourse.masks import make_identity

    dt = mybir.dt
    mult = mybir.AluOpType.mult
    add = mybir.AluOpType.add
    sub = mybir.AluOpType.subtract

    nc = bacc.Bacc("TRN2", target_bir_lowering=False, debug=False,
                   num_devices=NCORES)

    IDX = nc.dram_tensor("IDX", [P, 8], dt.int32, kind="ExternalInput").ap()
    E0G = nc.dram_tensor("E0G", [NENT, RANK], dt.float16, kind="ExternalInput").ap()
    RCAT = nc.dram_tensor("RCAT", [NREL, RC_W], dt.float16, kind="ExternalInput").ap()
    TCAT = nc.dram_tensor("TCAT", [NTIME, TC_W], dt.float16, kind="ExternalInput").ap()
    E0T = nc.dram_tensor("E0T", [RANK, NHALF], dt.float16, kind="ExternalInput").ap()
    OUT = nc.dram_tensor("OUT", [2 * P, NHALF], dt.float16, kind="ExternalOutput").ap()

    with tile.TileContext(nc) as tc:
        with (
            tc.tile_pool(name="const", bufs=1) as constp,
            tc.tile_pool(name="gath", bufs=1) as gp,
            tc.tile_pool(name="ew", bufs=1) as ew,
            tc.tile_pool(name="pst", bufs=1, space="PSUM") as pst,
            tc.tile_pool(name="psm", bufs=3, space="PSUM") as psm,
        ):
            # ---- idx on the sync HWDGE ring: first thing it issues.
            idxt = gp.tile([P, 8], dt.int32)
            nc.sync.dma_start(idxt[:], IDX[:])

            # ---- E0T half-table stream on the Activation HWDGE ring,
            # independent of everything: starts at t~=2us, done ~18us.
            e0t = constp.tile([RANK, NHALF], dt.float16)
            for c0 in range(0, NHALF, LOADCH):
                nc.scalar.dma_start(e0t[:, c0:c0 + LOADCH],
                                    E0T[:, c0:c0 + LOADCH])

            ident = constp.tile([P, P], dt.float16)
            make_identity(nc, ident[:])

            # ---- per-tile gathers on gpsimd SWDGE (RCAT/TCAT first so the
            # head's rule chain can start before the lhs rows land).
            lhs = [gp.tile([P, RANK], dt.float16, name=f"lhs{j}") for j in range(2)]
            r8 = [gp.tile([P, RC_W], dt.float16, name=f"r8{j}") for j in range(2)]
            t8 = [gp.tile([P, TC_W], dt.float16, name=f"t8{j}") for j in range(2)]

            def gathers(j):
                for dst, src, col in ((r8[j], RCAT, 1), (t8[j], TCAT, 2),
                                      (lhs[j], E0G, 0)):
                    nc.gpsimd.indirect_dma_start(
                        out=dst[:], out_offset=None, in_=src[:],
                        in_offset=bass.IndirectOffsetOnAxis(
                            ap=idxt[:, 4 * j + col:4 * j + col + 1],
                            axis=0))

            gathers(0)
            gathers(1)

            def VTT(out, a, b_, op):
                nc.vector.tensor_tensor(out=out, in0=a, in1=b_, op=op)

            def GTT(out, a, b_, op):
                nc.gpsimd.tensor_tensor(out=out, in0=a, in1=b_, op=op)

            # fp16 temporaries (vector-owned and gpsimd-owned kept separate)
            t0 = [ew.tile([P, 64], dt.float16, name=f"t0_{j}") for j in range(2)]
            t1 = [ew.tile([P, 64], dt.float16, name=f"t1_{j}") for j in range(2)]
            u0 = [ew.tile([P, 64], dt.float16, name=f"u0_{j}") for j in range(2)]
            u1 = [ew.tile([P, 64], dt.float16, name=f"u1_{j}") for j in range(2)]
            A = [ew.tile([P, RANK], dt.float16, name=f"A{j}") for j in range(2)]
            Bt = [ew.tile([P, RANK], dt.float16, name=f"B{j}") for j in range(2)]
            Sg = [ew.tile([P, RANK], dt.float16, name=f"S{j}") for j in range(2)]
            Dg = [ew.tile([P, RANK], dt.float16, name=f"D{j}") for j in range(2)]
            V = [ew.tile([P, RANK], dt.float16, name=f"V{j}") for j in range(2)]

            def head_gpsimd_rule(j):
                # A = cmul(CT, RC) - rule_S*REL   (NRS = -rule_S from host)
                REL = r8[j][:, 0:128]
                RC0, RC1 = r8[j][:, 128:192], r8[j][:, 192:256]
                NRS = r8[j][:, 256:257]
                CT0, CT1 = t8[j][:, 0:64], t8[j][:, 64:128]
                a, b_ = u0[j], u1[j]
                GTT(a[:], CT0, RC0, mult)
                GTT(b_[:], CT1, RC1, mult)
                GTT(A[j][:, 0:64], a[:], b_[:], sub)
                GTT(a[:], CT0, RC1, mult)
                GTT(b_[:], CT1, RC0, mult)
                GTT(A[j][:, 64:128], a[:], b_[:], add)

            def head_vector(j):
                REL = r8[j][:, 0:128]
                RL0, RL1 = r8[j][:, 0:64], r8[j][:, 64:128]
                NRS = r8[j][:, 256:257]
                HR = r8[j][:, 257:258]
                CT = t8[j][:, 0:128]
                TM = t8[j][:, 128:256]
                LHS = lhs[j][:]
                L0, L1 = lhs[j][:, 0:64], lhs[j][:, 64:128]
                a, b_ = t0[j], t1[j]
                Aj, Bj = A[j], Bt[j]
                # B = lhs + cmul(REL, LHS)
                VTT(a[:], RL0, L0, mult)
                VTT(b_[:], RL1, L1, mult)
                VTT(Bj[:, 0:64], a[:], b_[:], sub)
                VTT(a[:], RL0, L1, mult)
                VTT(b_[:], RL1, L0, mult)
                VTT(Bj[:, 64:128], a[:], b_[:], add)
                VTT(Bj[:], Bj[:], LHS, add)
                # A = rule_branch = cmul(CT,RC) - rule_S*rel  (NRS = -rule_S)
                nc.vector.scalar_tensor_tensor(
                    out=Aj[:], in0=REL, scalar=NRS, in1=Aj[:],
                    op0=mult, op1=add)
                # A = rule_score = B + HR*(A - B); q = A + CT
                VTT(Aj[:], Aj[:], Bj[:], sub)
                nc.vector.scalar_tensor_tensor(
                    out=Aj[:], in0=Aj[:], scalar=HR, in1=Bj[:],
                    op0=mult, op1=add)
                VTT(Aj[:], Aj[:], CT, add)
                # B = rel_ = REL + complex_mul(REL, q)
                q0, q1 = Aj[:, 0:64], Aj[:, 64:128]
                VTT(a[:], RL0, q0, mult)
                VTT(b_[:], RL1, q1, mult)
                VTT(Bj[:, 0:64], a[:], b_[:], add)
                VTT(a[:], RL0, q1, mult)
                VTT(b_[:], RL1, q0, mult)
                VTT(Bj[:, 64:128], a[:], b_[:], sub)
                VTT(Bj[:], Bj[:], REL, add)
                # S = rel_ + time ; D = rel_ - time
                VTT(Sg[j][:], Bj[:], TM, add)
                VTT(Dg[j][:], Bj[:], TM, sub)
                # V0 = L0*S0 + TE0*D0 - L1*S1 + TE1*D1
                TE0 = t8[j][:, 256:320]
                TE1 = t8[j][:, 320:384]
                S0, S1 = Sg[j][:, 0:64], Sg[j][:, 64:128]
                D0, D1 = Dg[j][:, 0:64], Dg[j][:, 64:128]
                V0 = V[j][:, 0:64]
                VTT(a[:], L0, S0, mult)
                VTT(b_[:], TE0, D0, mult)
                VTT(V0, a[:], b_[:], add)
                VTT(a[:], L1, S1, mult)
                VTT(V0, V0, a[:], sub)
                VTT(b_[:], TE1, D1, mult)
                VTT(V0, V0, b_[:], add)

            def head_gpsimd_v1(j):
                L0, L1 = lhs[j][:, 0:64], lhs[j][:, 64:128]
                TE0 = t8[j][:, 256:320]
                TE1 = t8[j][:, 320:384]
                S0, S1 = Sg[j][:, 0:64], Sg[j][:, 64:128]
                D0, D1 = Dg[j][:, 0:64], Dg[j][:, 64:128]
                V1 = V[j][:, 64:128]
                a, b_ = u0[j], u1[j]
                GTT(a[:], L1, S0, mult)
                GTT(b_[:], L0, S1, mult)
                GTT(V1, a[:], b_[:], add)
                GTT(a[:], TE1, D0, mult)
                GTT(V1, V1, a[:], add)
                GTT(b_[:], TE0, D1, mult)
                GTT(V1, V1, b_[:], sub)

            vts = []

            def finish_vt(j):
                vt_ps = pst.tile([P, P], dt.float16, space="PSUM", tag="vtps")
                nc.tensor.transpose(out=vt_ps[:], in_=V[j][:], identity=ident[:])
                vt = constp.tile([P, P], dt.float16, name=f"vt{j}")
                nc.scalar.copy(out=vt[:], in_=vt_ps[:])
                vts.append(vt)

            # gpsimd program: gathers0, gathers1 already queued; now rule
            # chains; V1 chains interleave with the vector heads.
            head_gpsimd_rule(0)
            head_vector(0)          # vector program: head0 ...
            head_gpsimd_rule(1)
            head_gpsimd_v1(0)
            finish_vt(0)

            # ---- stream matmuls + PSUM->SBUF copies (vector/scalar) + OUT
            # groups.  Each PSUM tile is [128,1000] f32 = 2 banks; the two
            # 500-col matmuls land in its two bank-aligned halves and one
            # copy instruction drains both.  Vector's first copy comes before
            # the tile-1 head; groups 1-3 go to scalar while vector runs that
            # head; afterwards the engines alternate.
            GRP = 2 * CHUNK          # 1024 columns per copy group
            osb = [constp.tile([P, NHALF], dt.float16, name=f"osb{j}")
                   for j in range(2)]
            head1_emitted = False
            g = 0
            for j in range(2):
                next_out = OUTCH
                for c0 in range(0, NHALF, GRP):
                    gw = min(GRP, NHALF - c0)
                    mm = psm.tile([P, GRP], dt.float32, space="PSUM", tag="mm")
                    for lo in range(0, gw, CHUNK):
                        cw = min(CHUNK, gw - lo)
                        nc.tensor.matmul(out=mm[:, lo:lo + cw],
                                         lhsT=vts[j][:],
                                         rhs=e0t[:, c0 + lo:c0 + lo + cw],
                                         start=True, stop=True)
                    if g == 0 or (g >= 4 and g % 2 == 0):
                        nc.vector.tensor_copy(out=osb[j][:, c0:c0 + gw],
                                              in_=mm[:, :gw])
                    else:
                        nc.scalar.copy(out=osb[j][:, c0:c0 + gw],
                                       in_=mm[:, :gw])
                    if not head1_emitted:
                        # vector program: head0, copy g0, head1, copies...
                        head_vector(1)
                        head_gpsimd_v1(1)
                        finish_vt(1)
                        head1_emitted = True
                    g += 1
                    if c0 + gw >= next_out or c0 + gw == NHALF:
                        o0 = next_out - OUTCH
                        hi = min(c0 + gw, NHALF)
                        nc.sync.dma_start(OUT[j * P:(j + 1) * P, o0:hi],
                                          osb[j][:, o0:hi])
                        next_out += OUTCH

    nc.compile()
    return nc


def _prep_inputs(inputs):
    x = np.asarray(inputs["x"])
    E0 = np.ascontiguousarray(np.asarray(inputs["E0"], dtype=np.float32))
    E1 = np.asarray(inputs["E1"], dtype=np.float32)
    E2 = np.asarray(inputs["E2"], dtype=np.float32)
    E3 = np.asarray(inputs["E3"], dtype=np.float32)
    E4 = np.asarray(inputs["E4"], dtype=np.float32)
    E5 = np.asarray(inputs["E5"], dtype=np.float32)
    E6 = np.asarray(inputs["E6"], dtype=np.float32)
    rule_C = np.asarray(inputs["rule_C"], dtype=np.float32)
    rule_S = np.asarray(inputs["rule_S"], dtype=np.float32)
    has_rules = np.asarray(inputs["has_rules"])

    idx = np.zeros((B, 4), np.int32)
    idx[:, 0] = x[:, 0]
    idx[:, 1] = x[:, 1]
    idx[:, 2] = x[:, 3]

    rcat = np.ascontiguousarray(np.concatenate(
        [E1, rule_C, -rule_S[:, None],
         has_rules.astype(np.float32)[:, None]], axis=1).astype(np.float16))
    tb = np.arange(NTIME) // CYCLE
    tcat = np.ascontiguousarray(np.concatenate(
        [E4, E2 + E5[tb], E3 + E6[tb]], axis=1).astype(np.float16))
    e0g = np.ascontiguousarray(E0.astype(np.float16))
    e0t = np.ascontiguousarray(E0.T).astype(np.float16)
    e0t_halves = [np.ascontiguousarray(e0t[:, :NHALF]),
                  np.ascontiguousarray(e0t[:, NHALF:])]

    in_maps = []
    for c in range(NCORES):
        p = c // 2
        idx2 = np.hstack([idx[2 * p * P:(2 * p + 1) * P],
                          idx[(2 * p + 1) * P:(2 * p + 2) * P]])
        in_maps.append({
            "IDX": np.ascontiguousarray(idx2),
            "E0G": e0g, "RCAT": rcat, "TCAT": tcat,
            "E0T": e0t_halves[c % 2],
        })
    return in_maps


def kernel(**inputs):
    from concourse.bass_utils import run_bass_kernel_spmd

    if "nc" not in _CACHE:
        _CACHE["nc"] = _build()
    nc = _CACHE["nc"]

    in_maps = _prep_inputs(inputs)
    res = run_bass_kernel_spmd(nc, in_maps, core_ids=list(range(NCORES)),
                               trace=TRACE)
    _CACHE["last_result"] = res
    out = np.empty((B, NENT), np.float32)
    for p in range(NCORES // 2):
        lo = res.results[2 * p]["OUT"]        # [256, 0:20000]
        hi = res.results[2 * p + 1]["OUT"]    # [256, 20000:40000]
        rows = slice(2 * p * P, (2 * p + 2) * P)
        out[rows, :NHALF] = lo
        out[rows, NHALF:] = hi
    return out


# revision 15
# speedup vs baseline: 1.0447x; 1.0192x over previous
"""Trainium2 Bass kernel for nn_CTRule (temporal KG scoring model).

Computes, for each of B=1024 queries (h, r, t):
  v = f(E0[h], E1[r], time tables, rule tables)   # [B, 128] elementwise algebra
  scores = v @ E0.T                               # [B, 40000]

Distribution over the 8 NeuronCores (pair-hybrid): the two cores of pair p
process batch tiles 2p, 2p+1 against disjoint halves of the 40000-entity
axis.  Per core: gather (indirect DMA) the three per-example table rows,
run the elementwise head in fp16 on VectorE+GpSimd, transpose v on TensorE,
stream this core's E0T half (fp16, 5.12MB) through 40-chunk matmuls per
tile, and write the fp16 [256, 20000] score block back to HBM.

Overlap structure (the point of this version vs the serialized baseline):
  * E0T chunk loads go on the Activation-engine HWDGE ring starting at t=0
    (no dependency on the index load).
  * idx goes on the Sync HWDGE ring first; gathers follow on gpsimd SWDGE.
  * OUT write groups go on the Sync ring as soon as each 2000-col group of
    PSUM->SBUF copies lands, so the 30us write stream overlaps the reads.
  * PSUM->SBUF copies are spread across Scalar/GpSimd/Vector.
Everything in the head is fp16 (2x DVE throughput; rel err ~1e-3 vs the
2e-2 gate).  No cross-core communication; the host reassembles 8 blocks.
"""

import numpy as np

P = 128
B = 1024
RANK = 128
NENT = 40000
NREL = 230
NTIME = 365
CYCLE = 120
NCORES = 8
NHALF = NENT // 2        # entity columns per core = 20000
CHUNK = 512              # matmul chunk columns (= one PSUM bank of f32)
LOADCH = 2500            # E0T load-chunk columns (8 loads of 0.64MB)
OUTCH = 2048             # output DMA group columns (2 copy groups)

RC_W = 2 * RANK + 2      # [E1 | rule_C | -rule_S | has_rules] = 258
TC_W = 3 * RANK          # [E4 | E2+E5b | E3+E6b] = 384

TRACE = False            # set by test harness for profiling runs
_CACHE = {}


def _build():
    import concourse.bass as bass
    import concourse.mybir as mybir
    import concourse.tile as tile
    from concourse import bacc
    from concourse.masks import make_identity

    dt = mybir.dt
    mult = mybir.AluOpType.mult
    add = mybir.AluOpType.add
    sub = mybir.AluOpType.subtract

    nc = bacc.Bacc("TRN2", target_bir_lowering=False, debug=False,
                   num_devices=NCORES)

    IDX = nc.dram_tensor("IDX", [P, 8], dt.int32, kind="ExternalInput").ap()
    E0G = nc.dram_tensor("E0G", [NENT, RANK], dt.float16, kind="ExternalInput").ap()
    RCAT = nc.dram_tensor("RCAT", [NREL, RC_W], dt.float16, kind="ExternalInput").ap()
    TCAT = nc.dram_tensor("TCAT", [NTIME, TC_W], dt.float16, kind="ExternalInput").ap()
    E0T = nc.dram_tensor("E0T", [RANK, NHALF], dt.float16, kind="ExternalInput").ap()
    OUT = nc.dram_tensor("OUT", [2 * P, NHALF], dt.float16, kind="ExternalOutput").ap()

    with tile.TileContext(nc) as tc:
        with (
            tc.tile_pool(name="const", bufs=1) as constp,
            tc.tile_pool(name="gath", bufs=1) as gp,
            tc.tile_pool(name="ew", bufs=1) as ew,
            tc.tile_pool(name="pst", bufs=1, space="PSUM") as pst,
            tc.tile_pool(name="psm", bufs=3, space="PSUM") as psm,
        ):
            # ---- idx first on the scalar HWDGE ring (it reaches its first
            # issue slot earliest), then the E0T chunk stream on the same
            # ring (no dependencies, saturates DMA while gathers trickle).
            idxt = gp.tile([P, 8], dt.int32)
            nc.scalar.dma_start(idxt[:], IDX[:])

            e0t = constp.tile([RANK, NHALF], dt.float16)
            for c0 in range(0, NHALF, LOADCH):
                nc.scalar.dma_start(e0t[:, c0:c0 + LOADCH],
                                    E0T[:, c0:c0 + LOADCH])

            ident = constp.tile([P, P], dt.float16)
            make_identity(nc, ident[:])

            # ---- gathers on gpsimd SWDGE, tile 0's three tables first so
            # its head starts ~3.4us before tile 1's tables even land.
            # idx layout: cols (r0,r1, t0,t1, h0,h1, pad,pad)
            lhsv = gp.tile([P, 2, RANK], dt.float16, name="lhs")
            r8v = gp.tile([P, 2, RC_W], dt.float16, name="r8")
            t8v = gp.tile([P, 2, TC_W], dt.float16, name="t8")
            for j in range(2):
                for dst, src, col in ((r8v, RCAT, 0), (t8v, TCAT, 2),
                                      (lhsv, E0G, 4)):
                    nc.gpsimd.indirect_dma_start(
                        out=dst[:, j, :], out_offset=None, in_=src[:],
                        in_offset=bass.IndirectOffsetOnAxis(
                            ap=idxt[:, col + j:col + j + 1], axis=0))
            r8 = [r8v[:, j, :] for j in range(2)]
            t8 = [t8v[:, j, :] for j in range(2)]
            lhs = [lhsv[:, j, :] for j in range(2)]

            def VTT(out, a, b_, op):
                nc.vector.tensor_tensor(out=out, in0=a, in1=b_, op=op)

            def GTT(out, a, b_, op):
                nc.gpsimd.tensor_tensor(out=out, in0=a, in1=b_, op=op)

            # fp16 temporaries (vector-owned and gpsimd-owned kept separate)
            t0 = [ew.tile([P, 64], dt.float16, name=f"t0_{j}") for j in range(2)]
            t1 = [ew.tile([P, 64], dt.float16, name=f"t1_{j}") for j in range(2)]
            u0 = [ew.tile([P, 64], dt.float16, name=f"u0_{j}") for j in range(2)]
            u1 = [ew.tile([P, 64], dt.float16, name=f"u1_{j}") for j in range(2)]
            A = [ew.tile([P, RANK], dt.float16, name=f"A{j}") for j in range(2)]
            Bt = [ew.tile([P, RANK], dt.float16, name=f"B{j}") for j in range(2)]
            Sg = [ew.tile([P, RANK], dt.float16, name=f"S{j}") for j in range(2)]
            Dg = [ew.tile([P, RANK], dt.float16, name=f"D{j}") for j in range(2)]
            V = [ew.tile([P, RANK], dt.float16, name=f"V{j}") for j in range(2)]

            def head_gpsimd_rule(j):
                # A = cmul(CT, RC) - rule_S*REL   (NRS = -rule_S from host)
                REL = r8[j][:, 0:128]
                RC0, RC1 = r8[j][:, 128:192], r8[j][:, 192:256]
                NRS = r8[j][:, 256:257]
                CT0, CT1 = t8[j][:, 0:64], t8[j][:, 64:128]
                a, b_ = u0[j], u1[j]
                GTT(a[:], CT0, RC0, mult)
                GTT(b_[:], CT1, RC1, mult)
                GTT(A[j][:, 0:64], a[:], b_[:], sub)
                GTT(a[:], CT0, RC1, mult)
                GTT(b_[:], CT1, RC0, mult)
                GTT(A[j][:, 64:128], a[:], b_[:], add)

            def head_vector(j):
                REL = r8[j][:, 0:128]
                RL0, RL1 = r8[j][:, 0:64], r8[j][:, 64:128]
                NRS = r8[j][:, 256:257]
                HR = r8[j][:, 257:258]
                CT = t8[j][:, 0:128]
                TM = t8[j][:, 128:256]
                LHS = lhs[j][:]
                L0, L1 = lhs[j][:, 0:64], lhs[j][:, 64:128]
                a, b_ = t0[j], t1[j]
                Aj, Bj = A[j], Bt[j]
                # B = lhs + cmul(REL, LHS)
                VTT(a[:], RL0, L0, mult)
                VTT(b_[:], RL1, L1, mult)
                VTT(Bj[:, 0:64], a[:], b_[:], sub)
                VTT(a[:], RL0, L1, mult)
                VTT(b_[:], RL1, L0, mult)
                VTT(Bj[:, 64:128], a[:], b_[:], add)
                VTT(Bj[:], Bj[:], LHS, add)
                # A = rule_branch = cmul(CT,RC) - rule_S*rel  (NRS = -rule_S)
                nc.vector.scalar_tensor_tensor(
                    out=Aj[:], in0=REL, scalar=NRS, in1=Aj[:],
                    op0=mult, op1=add)
                # A = rule_score = B + HR*(A - B); q = A + CT
                VTT(Aj[:], Aj[:], Bj[:], sub)
                nc.vector.scalar_tensor_tensor(
                    out=Aj[:], in0=Aj[:], scalar=HR, in1=Bj[:],
                    op0=mult, op1=add)
                VTT(Aj[:], Aj[:], CT, add)
                # B = rel_ = REL + complex_mul(REL, q)
                q0, q1 = Aj[:, 0:64], Aj[:, 64:128]
                VTT(a[:], RL0, q0, mult)
                VTT(b_[:], RL1, q1, mult)
                VTT(Bj[:, 0:64], a[:], b_[:], add)
                VTT(a[:], RL0, q1, mult)
                VTT(b_[:], RL1, q0, mult)
                VTT(Bj[:, 64:128], a[:], b_[:], sub)
                VTT(Bj[:], Bj[:], REL, add)
                # S = rel_ + time ; D = rel_ - time
                VTT(Sg[j][:], Bj[:], TM, add)
                VTT(Dg[j][:], Bj[:], TM, sub)
                # V0 = L0*S0 + TE0*D0 - L1*S1 + TE1*D1
                TE0 = t8[j][:, 256:320]
                TE1 = t8[j][:, 320:384]
                S0, S1 = Sg[j][:, 0:64], Sg[j][:, 64:128]
                D0, D1 = Dg[j][:, 0:64], Dg[j][:, 64:128]
                V0 = V[j][:, 0:64]
                VTT(a[:], L0, S0, mult)
                VTT(b_[:], TE0, D0, mult)
                VTT(V0, a[:], b_[:], add)
                VTT(a[:], L1, S1, mult)
                VTT(V0, V0, a[:], sub)
                VTT(b_[:], TE1, D1, mult)
                VTT(V0, V0, b_[:], add)

            def head_gpsimd_v1(j):
                L0, L1 = lhs[j][:, 0:64], lhs[j][:, 64:128]
                TE0 = t8[j][:, 256:320]
                TE1 = t8[j][:, 320:384]
                S0, S1 = Sg[j][:, 0:64], Sg[j][:, 64:128]
                D0, D1 = Dg[j][:, 0:64], Dg[j][:, 64:128]
                V1 = V[j][:, 64:128]
                a, b_ = u0[j], u1[j]
                GTT(a[:], L1, S0, mult)
                GTT(b_[:], L0, S1, mult)
                GTT(V1, a[:], b_[:], add)
                GTT(a[:], TE1, D0, mult)
                GTT(V1, V1, a[:], add)
                GTT(b_[:], TE0, D1, mult)
                GTT(V1, V1, b_[:], sub)

            vts = []

            def finish_vt(j):
                vt_ps = pst.tile([P, P], dt.float16, space="PSUM", tag="vtps")
                nc.tensor.transpose(out=vt_ps[:], in_=V[j][:], identity=ident[:])
                vt = constp.tile([P, P], dt.float16, name=f"vt{j}")
                nc.scalar.copy(out=vt[:], in_=vt_ps[:])
                vts.append(vt)

            # gpsimd program: gathers0, gathers1 already queued; now rule
            # chains; V1 chains interleave with the vector heads.
            head_gpsimd_rule(0)
            head_vector(0)          # vector program: head0 ...
            head_gpsimd_rule(1)
            head_gpsimd_v1(0)
            finish_vt(0)

            # ---- stream matmuls + PSUM->SBUF copies (vector/scalar) + OUT
            # groups.  Each PSUM tile is [128,1000] f32 = 2 banks; the two
            # 500-col matmuls land in its two bank-aligned halves and one
            # copy instruction drains both.  Vector's first copy comes before
            # the tile-1 head; groups 1-3 go to scalar while vector runs that
            # head; afterwards the engines alternate.
            GRP = 2 * CHUNK          # 1024 columns per copy group
            osb = [constp.tile([P, NHALF], dt.float16, name=f"osb{j}")
                   for j in range(2)]
            head1_emitted = False
            g = 0
            for j in range(2):
                next_out = OUTCH
                for c0 in range(0, NHALF, GRP):
                    gw = min(GRP, NHALF - c0)
                    mm = psm.tile([P, GRP], dt.float32, space="PSUM", tag="mm")
                    for lo in range(0, gw, CHUNK):
                        cw = min(CHUNK, gw - lo)
                        nc.tensor.matmul(out=mm[:, lo:lo + cw],
                                         lhsT=vts[j][:],
                                         rhs=e0t[:, c0 + lo:c0 + lo + cw],
                                         start=True, stop=True)
                    if g == 0 or (g >= 4 and g % 2 == 0):
                        nc.vector.tensor_copy(out=osb[j][:, c0:c0 + gw],
                                              in_=mm[:, :gw])
                    else:
                        nc.scalar.copy(out=osb[j][:, c0:c0 + gw],
                                       in_=mm[:, :gw])
                    if not head1_emitted:
                        # vector program: head0, copy g0, head1, copies...
                        head_vector(1)
                        head_gpsimd_v1(1)
                        head1_emitted = True
                    elif g == 6:
                        # tile-1 transpose emitted a few groups in, so the
                        # tensor stream is never parked waiting on head 1
                        finish_vt(1)
                    g += 1
                    if c0 + gw >= next_out or c0 + gw == NHALF:
                        o0 = next_out - OUTCH
                        hi = min(c0 + gw, NHALF)
                        nc.sync.dma_start(OUT[j * P:(j + 1) * P, o0:hi],
                                          osb[j][:, o0:hi])
                        next_out += OUTCH

    nc.compile()
    return nc


def _prep_inputs(inputs):
    x = np.asarray(inputs["x"])
    E0 = np.ascontiguousarray(np.asarray(inputs["E0"], dtype=np.float32))
    E1 = np.asarray(inputs["E1"], dtype=np.float32)
    E2 = np.asarray(inputs["E2"], dtype=np.float32)
    E3 = np.asarray(inputs["E3"], dtype=np.float32)
    E4 = np.asarray(inputs["E4"], dtype=np.float32)
    E5 = np.asarray(inputs["E5"], dtype=np.float32)
    E6 = np.asarray(inputs["E6"], dtype=np.float32)
    rule_C = np.asarray(inputs["rule_C"], dtype=np.float32)
    rule_S = np.asarray(inputs["rule_S"], dtype=np.float32)
    has_rules = np.asarray(inputs["has_rules"])

    idx = np.zeros((B, 4), np.int32)
    idx[:, 0] = x[:, 1]    # r
    idx[:, 1] = x[:, 3]    # t
    idx[:, 2] = x[:, 0]    # h

    rcat = np.ascontiguousarray(np.concatenate(
        [E1, rule_C, -rule_S[:, None],
         has_rules.astype(np.float32)[:, None]], axis=1).astype(np.float16))
    tb = np.arange(NTIME) // CYCLE
    tcat = np.ascontiguousarray(np.concatenate(
        [E4, E2 + E5[tb], E3 + E6[tb]], axis=1).astype(np.float16))
    e0g = np.ascontiguousarray(E0.astype(np.float16))
    e0t = np.ascontiguousarray(E0.T).astype(np.float16)
    e0t_halves = [np.ascontiguousarray(e0t[:, :NHALF]),
                  np.ascontiguousarray(e0t[:, NHALF:])]

    in_maps = []
    for c in range(NCORES):
        p = c // 2
        i0 = idx[2 * p * P:(2 * p + 1) * P]        # tile 0 (r,t,h,pad)
        i1 = idx[(2 * p + 1) * P:(2 * p + 2) * P]  # tile 1
        # interleave to (r0,r1, t0,t1, h0,h1, pad,pad)
        idx2 = np.empty((P, 8), np.int32)
        idx2[:, 0::2] = i0
        idx2[:, 1::2] = i1
        in_maps.append({
            "IDX": np.ascontiguousarray(idx2),
            "E0G": e0g, "RCAT": rcat, "TCAT": tcat,
            "E0T": e0t_halves[c % 2],
        })
    return in_maps


def kernel(**inputs):
    from concourse.bass_utils import run_bass_kernel_spmd

    if "nc" not in _CACHE:
        _CACHE["nc"] = _build()
    nc = _CACHE["nc"]

    in_maps = _prep_inputs(inputs)
    res = run_bass_kernel_spmd(nc, in_maps, core_ids=list(range(NCORES)),
                               trace=TRACE)
    _CACHE["last_result"] = res
    out = np.empty((B, NENT), np.float32)
    for p in range(NCORES // 2):
        lo = res.results[2 * p]["OUT"]        # [256, 0:20000]
        hi = res.results[2 * p + 1]["OUT"]    # [256, 20000:40000]
        rows = slice(2 * p * P, (2 * p + 2) * P)
        out[rows, :NHALF] = lo
        out[rows, NHALF:] = hi
    return out


# revision 20
# speedup vs baseline: 1.0898x; 1.0432x over previous
"""Trainium2 Bass kernel for nn_CTRule (temporal KG scoring model).

Computes, for each of B=1024 queries (h, r, t):
  v = f(E0[h], E1[r], time tables, rule tables)   # [B, 128] elementwise algebra
  scores = v @ E0.T                               # [B, 40000]

Distribution over the 8 NeuronCores (pair-hybrid): the two cores of pair p
process batch tiles 2p, 2p+1 against disjoint halves of the 40000-entity
axis.  Per core: gather (indirect DMA) the per-example table rows, run the
elementwise head in fp16 on VectorE (+GpSimd for the independent rule
chain), transpose v on TensorE, stream this core's E0T half through
40-chunk matmuls per tile, and write the fp16 [256, 20000] block to HBM.

Latency structure (from trace analysis of prior versions):
  * idx is DMA'd by gpsimd itself (lands ~3us; the HWDGE rings only reach
    their first issue slot at ~5-7us after library loads).
  * gathers issue back-to-back on gpsimd right after idx; tile 0's tables
    first.  Tables are host-augmented with swapped halves ([x0|x1]->[x1|x0])
    so every complex-product pair is ONE wide [P,256] fp16 multiply.
  * the head is a single-engine chain on Vector (no cross-engine ping-pong)
    except the rule cmul which GpSimd computes concurrently.
  * E0T chunk loads run on the Activation HWDGE ring from ~7us (no deps).
  * matmul chunks are 512 cols (one PSUM bank); pairs share a [P,1024] PSUM
    tile drained by one copy (Vector/Scalar alternate); every 1024-col group
    is DMA'd to HBM on the Sync ring as soon as its copy lands, so the
    ~31us write stream overlaps everything else.
All head math in fp16 (rel err ~6e-4 total vs the 2e-2 gate).  No
cross-core communication; the host reassembles the 8 blocks.
"""

import numpy as np

P = 128
B = 1024
RANK = 128
NENT = 40000
NREL = 230
NTIME = 365
CYCLE = 120
NCORES = 8
NHALF = NENT // 2        # entity columns per core = 20000
CHUNK = 512              # matmul chunk columns (= one PSUM bank of f32)
LOADCH = 2500            # E0T load-chunk columns (8 loads of 0.64MB)
OUTCH = 1024             # output DMA group columns (= one copy group)

RC_W = 4 * RANK + 2      # [E1 | E1sw | rule_C | rule_Csw | -rS | hr] = 514
TC_W = 5 * RANK          # [E4 | E4 | TM | TE | TEsw] = 640
LH_W = 2 * RANK          # [E0row | E0row-swapped] = 256

TRACE = False            # set by test harness for profiling runs
_CACHE = {}


def _build():
    import concourse.bass as bass
    import concourse.mybir as mybir
    import concourse.tile as tile
    from concourse import bacc
    from concourse.masks import make_identity

    dt = mybir.dt
    mult = mybir.AluOpType.mult
    add = mybir.AluOpType.add
    sub = mybir.AluOpType.subtract

    nc = bacc.Bacc("TRN2", target_bir_lowering=False, debug=False,
                   num_devices=NCORES)

    IDX = nc.dram_tensor("IDX", [P, 8], dt.int32, kind="ExternalInput").ap()
    E0G = nc.dram_tensor("E0G", [NENT, LH_W], dt.float16, kind="ExternalInput").ap()
    RCAT = nc.dram_tensor("RCAT", [NREL, RC_W], dt.float16, kind="ExternalInput").ap()
    TCAT = nc.dram_tensor("TCAT", [NTIME, TC_W], dt.float16, kind="ExternalInput").ap()
    E0T = nc.dram_tensor("E0T", [RANK, NHALF], dt.float16, kind="ExternalInput").ap()
    OUT = nc.dram_tensor("OUT", [2 * P, NHALF], dt.float16, kind="ExternalOutput").ap()

    with tile.TileContext(nc) as tc:
        with (
            tc.tile_pool(name="const", bufs=1) as constp,
            tc.tile_pool(name="gath", bufs=1) as gp,
            tc.tile_pool(name="ew", bufs=1) as ew,
            tc.tile_pool(name="pst", bufs=1, space="PSUM") as pst,
            tc.tile_pool(name="psm", bufs=3, space="PSUM") as psm,
        ):
            # ---- idx via gpsimd's own SWDGE: lands ~3us, and the gather
            # issue chain on the same engine follows immediately.
            idxt = gp.tile([P, 8], dt.int32)
            nc.gpsimd.dma_start(idxt[:], IDX[:])

            # idx layout: cols (r0,r1, t0,t1, h0,h1, pad,pad); tile0 first
            lhsv = gp.tile([P, 2, LH_W], dt.float16, name="lhs")
            r8v = gp.tile([P, 2, RC_W], dt.float16, name="r8")
            t8v = gp.tile([P, 2, TC_W], dt.float16, name="t8")
            for j in range(2):
                for dst, src, col in ((r8v, RCAT, 0), (t8v, TCAT, 2),
                                      (lhsv, E0G, 4)):
                    nc.gpsimd.indirect_dma_start(
                        out=dst[:, j, :], out_offset=None, in_=src[:],
                        in_offset=bass.IndirectOffsetOnAxis(
                            ap=idxt[:, col + j:col + j + 1], axis=0))

            # ---- E0T half-table stream on the Activation HWDGE ring
            e0t = constp.tile([RANK, NHALF], dt.float16)
            for c0 in range(0, NHALF, LOADCH):
                nc.scalar.dma_start(e0t[:, c0:c0 + LOADCH],
                                    E0T[:, c0:c0 + LOADCH])

            ident = constp.tile([P, P], dt.float16)
            make_identity(nc, ident[:])

            def VTT(out, a, b_, op):
                nc.vector.tensor_tensor(out=out, in0=a, in1=b_, op=op)

            def GTT(out, a, b_, op):
                nc.gpsimd.tensor_tensor(out=out, in0=a, in1=b_, op=op)

            A = [ew.tile([P, RANK], dt.float16, name=f"A{j}") for j in range(2)]
            Bt = [ew.tile([P, RANK], dt.float16, name=f"B{j}") for j in range(2)]
            PB = [ew.tile([P, 2 * RANK], dt.float16, name=f"PB{j}") for j in range(2)]
            QQ = [ew.tile([P, 2 * RANK], dt.float16, name=f"QQ{j}") for j in range(2)]
            SS = [ew.tile([P, 2 * RANK], dt.float16, name=f"SS{j}") for j in range(2)]
            DD = [ew.tile([P, 2 * RANK], dt.float16, name=f"DD{j}") for j in range(2)]
            PL = [ew.tile([P, 2 * RANK], dt.float16, name=f"PL{j}") for j in range(2)]
            PT = [ew.tile([P, 2 * RANK], dt.float16, name=f"PT{j}") for j in range(2)]
            t0 = [ew.tile([P, 64], dt.float16, name=f"t0_{j}") for j in range(2)]
            t1 = [ew.tile([P, 64], dt.float16, name=f"t1_{j}") for j in range(2)]
            V = [ew.tile([P, RANK], dt.float16, name=f"V{j}") for j in range(2)]

            def head_gpsimd_rule(j):
                # A = cmul(CT, RC):  PA = [CT|CT]*[RC|RCsw] then halves.
                # PA = [CT0RC0|CT1RC1 | CT0RC1|CT1RC0]
                PA = ew.tile([P, 2 * RANK], dt.float16, name=f"PA{j}")
                GTT(PA[:], t8v[:, j, 0:256], r8v[:, j, 256:512], mult)
                GTT(A[j][:, 0:64], PA[:, 0:64], PA[:, 64:128], sub)
                GTT(A[j][:, 64:128], PA[:, 128:192], PA[:, 192:256], add)

            def head_vector(j):
                r8 = r8v[:, j, :]
                t8 = t8v[:, j, :]
                lhs = lhsv[:, j, :]
                RELRELSW = r8[:, 0:256]
                REL = r8[:, 0:128]
                NRS = r8[:, 512:513]
                HR = r8[:, 513:514]
                CT = t8[:, 0:128]
                TM = t8[:, 256:384]
                TESW2 = t8[:, 384:640]
                LHS = lhs[:, 0:128]
                Aj, Bj = A[j], Bt[j]
                # B = lhs + cmul(REL, LHS):
                # PB = [REL|RELsw]*[L|L] = [RL0L0|RL1L1 | RL1L0|RL0L1]
                VTT(PB[j][:], RELRELSW, lhs[:, 0:256], mult)
                VTT(Bj[:, 0:64], PB[j][:, 0:64], PB[j][:, 64:128], sub)
                VTT(Bj[:, 64:128], PB[j][:, 128:192], PB[j][:, 192:256], add)
                VTT(Bj[:], Bj[:], LHS, add)
                # A = rule_branch = cmul(CT,RC) - rule_S*rel  (NRS = -rule_S)
                nc.vector.scalar_tensor_tensor(
                    out=Aj[:], in0=REL, scalar=NRS, in1=Aj[:],
                    op0=mult, op1=add)
                # A = rule_score = B + HR*(A - B); qq = [A+CT | A+CT]
                VTT(Aj[:], Aj[:], Bj[:], sub)
                nc.vector.scalar_tensor_tensor(
                    out=Aj[:], in0=Aj[:], scalar=HR, in1=Bj[:],
                    op0=mult, op1=add)
                VTT(QQ[j][:, 0:128], Aj[:], CT, add)
                VTT(QQ[j][:, 128:256], Aj[:], CT, add)
                # C = rel_ = REL + complex_mul(REL, q)
                # PC = [REL|RELsw]*[q|q] = [RL0q0|RL1q1 | RL1q0|RL0q1]
                PC = PB[j]
                VTT(PC[:], RELRELSW, QQ[j][:], mult)
                VTT(Bj[:, 0:64], PC[:, 0:64], PC[:, 64:128], add)
                VTT(Bj[:, 64:128], PC[:, 192:256], PC[:, 128:192], sub)
                VTT(Bj[:], Bj[:], REL, add)
                # SS = [S|Ssw], DD = [D|D] with S = rel_+time, D = rel_-time
                TM0 = t8[:, 256:320]
                TM1 = t8[:, 320:384]
                VTT(SS[j][:, 0:128], Bj[:], TM, add)
                VTT(SS[j][:, 128:192], Bj[:, 64:128], TM1, add)
                VTT(SS[j][:, 192:256], Bj[:, 0:64], TM0, add)
                VTT(DD[j][:, 0:128], Bj[:], TM, sub)
                VTT(DD[j][:, 128:256], Bj[:], TM, sub)
                # PL = [L|L]*[S|Ssw] = [L0S0|L1S1 | L0S1|L1S0]
                # PT = [TE|TEsw]*[D|D] = [TE0D0|TE1D1 | TE1D0|TE0D1]
                VTT(PL[j][:], lhs[:, 0:256], SS[j][:], mult)
                VTT(PT[j][:], TESW2, DD[j][:], mult)
                # V0 = (L0S0 - L1S1) + (TE0D0 + TE1D1)
                VTT(t0[j][:], PL[j][:, 0:64], PL[j][:, 64:128], sub)
                VTT(t1[j][:], PT[j][:, 0:64], PT[j][:, 64:128], add)
                VTT(V[j][:, 0:64], t0[j][:], t1[j][:], add)
                # V1 = (L0S1 + L1S0) + (TE1D0 - TE0D1)
                VTT(t0[j][:], PL[j][:, 128:192], PL[j][:, 192:256], add)
                VTT(t1[j][:], PT[j][:, 128:192], PT[j][:, 192:256], sub)
                VTT(V[j][:, 64:128], t0[j][:], t1[j][:], add)

            vts = []

            def finish_vt(j):
                vt_ps = pst.tile([P, P], dt.float16, space="PSUM", tag="vtps")
                nc.tensor.transpose(out=vt_ps[:], in_=V[j][:], identity=ident[:])
                vt = constp.tile([P, P], dt.float16, name=f"vt{j}")
                nc.scalar.copy(out=vt[:], in_=vt_ps[:])
                vts.append(vt)

            head_gpsimd_rule(0)
            head_gpsimd_rule(1)
            head_vector(0)
            finish_vt(0)

            # ---- stream matmuls + PSUM->SBUF copies + per-1024-col OUT DMAs
            GRP = 2 * CHUNK
            osb = [constp.tile([P, NHALF], dt.float16, name=f"osb{j}")
                   for j in range(2)]
            head1_emitted = False
            g = 0
            for j in range(2):
                for c0 in range(0, NHALF, GRP):
                    gw = min(GRP, NHALF - c0)
                    mm = psm.tile([P, GRP], dt.float32, space="PSUM", tag="mm")
                    for lo in range(0, gw, CHUNK):
                        cw = min(CHUNK, gw - lo)
                        nc.tensor.matmul(out=mm[:, lo:lo + cw],
                                         lhsT=vts[j][:],
                                         rhs=e0t[:, c0 + lo:c0 + lo + cw],
                                         start=True, stop=True)
                    if g == 0 or (g >= 7 and g % 2 == 1):
                        nc.vector.tensor_copy(out=osb[j][:, c0:c0 + gw],
                                              in_=mm[:, :gw])
                    else:
                        nc.scalar.copy(out=osb[j][:, c0:c0 + gw],
                                       in_=mm[:, :gw])
                    if not head1_emitted:
                        # vector program: head0, copy g0, head1, copies...
                        head_vector(1)
                        head1_emitted = True
                    elif g == 5:
                        # tile-1 transpose a few groups in: the tensor
                        # stream never parks waiting on head 1
                        finish_vt(1)
                    g += 1
                    nc.sync.dma_start(OUT[j * P:(j + 1) * P, c0:c0 + gw],
                                      osb[j][:, c0:c0 + gw])

    nc.compile()
    return nc


def _prep_inputs(inputs):
    x = np.asarray(inputs["x"])
    E0 = np.ascontiguousarray(np.asarray(inputs["E0"], dtype=np.float32))
    E1 = np.asarray(inputs["E1"], dtype=np.float32)
    E2 = np.asarray(inputs["E2"], dtype=np.float32)
    E3 = np.asarray(inputs["E3"], dtype=np.float32)
    E4 = np.asarray(inputs["E4"], dtype=np.float32)
    E5 = np.asarray(inputs["E5"], dtype=np.float32)
    E6 = np.asarray(inputs["E6"], dtype=np.float32)
    rule_C = np.asarray(inputs["rule_C"], dtype=np.float32)
    rule_S = np.asarray(inputs["rule_S"], dtype=np.float32)
    has_rules = np.asarray(inputs["has_rules"])

    idx = np.zeros((B, 4), np.int32)
    idx[:, 0] = x[:, 1]    # r
    idx[:, 1] = x[:, 3]    # t
    idx[:, 2] = x[:, 0]    # h

    def sw(a):
        return np.concatenate([a[:, RANK // 2:], a[:, :RANK // 2]], axis=1)

    rcat = np.ascontiguousarray(np.concatenate(
        [E1, sw(E1), rule_C, sw(rule_C), -rule_S[:, None],
         has_rules.astype(np.float32)[:, None]], axis=1).astype(np.float16))
    tb = np.arange(NTIME) // CYCLE
    TM = E2 + E5[tb]
    TE = E3 + E6[tb]
    tcat = np.ascontiguousarray(np.concatenate(
        [E4, E4, TM, TE, sw(TE)], axis=1).astype(np.float16))
    e0h = E0.astype(np.float16)
    e0g = np.ascontiguousarray(np.concatenate([e0h, e0h], axis=1))
    e0t = np.ascontiguousarray(E0.T).astype(np.float16)
    e0t_halves = [np.ascontiguousarray(e0t[:, :NHALF]),
                  np.ascontiguousarray(e0t[:, NHALF:])]

    in_maps = []
    for c in range(NCORES):
        p = c // 2
        i0 = idx[2 * p * P:(2 * p + 1) * P]        # tile 0 (r,t,h,pad)
        i1 = idx[(2 * p + 1) * P:(2 * p + 2) * P]  # tile 1
        idx2 = np.empty((P, 8), np.int32)
        idx2[:, 0::2] = i0
        idx2[:, 1::2] = i1
        in_maps.append({
            "IDX": np.ascontiguousarray(idx2),
            "E0G": e0g, "RCAT": rcat, "TCAT": tcat,
            "E0T": e0t_halves[c % 2],
        })
    return in_maps


def kernel(**inputs):
    from concourse.bass_utils import run_bass_kernel_spmd

    if "nc" not in _CACHE:
        _CACHE["nc"] = _build()
    nc = _CACHE["nc"]

    in_maps = _prep_inputs(inputs)
    res = run_bass_kernel_spmd(nc, in_maps, core_ids=list(range(NCORES)),
                               trace=TRACE)
    _CACHE["last_result"] = res
    out = np.empty((B, NENT), np.float32)
    for p in range(NCORES // 2):
        lo = res.results[2 * p]["OUT"]        # [256, 0:20000]
        hi = res.results[2 * p + 1]["OUT"]    # [256, 20000:40000]
        rows = slice(2 * p * P, (2 * p + 2) * P)
        out[rows, :NHALF] = lo
        out[rows, NHALF:] = hi
    return out
